# revision 1
# baseline (speedup 1.0000x reference)
"""AttentionBlock kernel for Trainium2, 8 NeuronCores.

Reference computation (B=4, C=256, H=W=64, TEMB=1024):
    t  = temb @ t_w.T + t_b                       # [B, C]
    q  = q_w @ x + (q_b + t)   (1x1 conv)         # [B, C, HW]
    k  = k_w @ x + (k_b + t)
    v  = v_w @ x + v_b
    att = softmax(q.T k / sqrt(C), axis=j)        # [B, HW, HW]
    hh  = att @ v.T                               # [B, C, HW]
    out = x + p_w @ hh + p_b

Sharding: data-parallel over (batch, query-half): core = b*2 + h.
Each core receives x[b] with its OWN query half rotated to the front
(keys may be processed in any order -- softmax is a sum over keys), so
one [C, HW] tensor serves the K/V convs, the Q conv (chunks 0-3) and
the residual.  The flash-attention-style kernel runs entirely in SBUF:
the 67M-entry attention matrix is never materialized to HBM.

Layout choices (evidence-driven from NTFF traces):
  - q/k convs and attention matmul 1 in float32r (FP22 multiply, FP32
    accumulate): full PE rate at free-dim >= 256, ~1e-4 rel error.
  - attention computed transposed: attT[j, i] = k[:,j] . q_scaled[:,i];
    exp on the scalar engine writes p as bf16 (error ~2e-5 measured).
  - matmul 2: hhT[i, c] = sum_j p[j,i] vT[j,c] with bf16 p-slices as
    weights (fast FWL loads) and bf16 vT as the 258-wide moving operand.
    vT carries a ones column, so hhT[:, 256] = softmax denominator
    lands per-partition for free; normalization is a per-partition
    scale on the DVE.
  - hhT -> channel-major hh via 8 bf16 PE transposes per query block
    (DMA transposes serialized ~1.2us each and stalled the PE).
  - block N's p-conv is deferred into block N+1's matmul stream, and
    query block 0's whole attention loop is interleaved with the convs
    chunk-by-chunk, so the PE chases the x DMA stream and the HAM clock
    gate never re-throttles mid-kernel.
"""

import numpy as np
import ml_dtypes
from contextlib import ExitStack

import concourse.bass as bass
import concourse.mybir as mybir
import concourse.tile as tile
from concourse import bacc
from concourse.bass_utils import run_bass_kernel_spmd

F32 = mybir.dt.float32
F32R = mybir.dt.float32r
BF16 = mybir.dt.bfloat16
AF = mybir.ActivationFunctionType

B, C, H, W, TEMB = 4, 256, 64, 64, 1024
HW = H * W              # 4096
NQ = HW // 2            # 2048 query pixels per core
N_CORES = 8
SCALE = float(C) ** -0.5

N_CH = HW // 512        # 8 x-chunks of 512 columns
N_JT = HW // 128        # 32 key tiles of 128
N_IB = NQ // 512        # 4 query blocks of 512
N_KT = C // 128         # 2 channel tiles
CV = C + 2              # vT width: 256 channels + ones col + pad


def build_nc():
    nc = bacc.Bacc("TRN2", target_bir_lowering=False, debug=False)

    # ---- DRAM I/O (per-core) ----
    xk_d = nc.dram_tensor("xk", [N_CH * C, 512], F32R, kind="ExternalInput")
    temb_d = nc.dram_tensor("tembc", [128, TEMB // 128], BF16, kind="ExternalInput")
    qwT_d = nc.dram_tensor("qwT", [C, C], F32R, kind="ExternalInput")
    kwT_d = nc.dram_tensor("kwT", [C, C], F32R, kind="ExternalInput")
    vwT_d = nc.dram_tensor("vwT", [C, CV], F32R, kind="ExternalInput")
    pwT_d = nc.dram_tensor("pwT", [C, C], BF16, kind="ExternalInput")
    twT_d = nc.dram_tensor("twT", [TEMB, C], BF16, kind="ExternalInput")
    qb_d = nc.dram_tensor("qb", [C, 1], F32, kind="ExternalInput")
    kb_d = nc.dram_tensor("kb", [C, 1], F32, kind="ExternalInput")
    vb_d = nc.dram_tensor("vb", [1, CV], F32, kind="ExternalInput")
    pb_d = nc.dram_tensor("pb", [C, 1], F32, kind="ExternalInput")
    tb_d = nc.dram_tensor("tb", [C, 1], F32, kind="ExternalInput")
    id_d = nc.dram_tensor("ident", [128, 128], BF16, kind="ExternalInput")
    out_d = nc.dram_tensor("out", [C, NQ], F32, kind="ExternalOutput")

    with tile.TileContext(nc) as tc, ExitStack() as ctx:
        const = ctx.enter_context(tc.tile_pool(name="const", bufs=1))
        big = ctx.enter_context(tc.tile_pool(name="big", bufs=1))

        # ---- loads, in rough order of first use on the PE ----
        def load3(dram, kt_n, width, name, dt):
            t = const.tile([128, kt_n, width], dt, tag=name)
            nc.sync.dma_start(out=t, in_=dram[:].rearrange("(a p) o -> p a o", p=128))
            return t

        def loadb(dram, name):
            t = const.tile([128, N_KT], F32, tag=name)
            nc.sync.dma_start(
                out=t, in_=dram[:].rearrange("(a p) one -> p (a one)", p=128)
            )
            return t

        twT = load3(twT_d, TEMB // 128, C, "twT", BF16)
        temb_sb = const.tile([128, TEMB // 128], BF16, tag="temb")
        nc.sync.dma_start(out=temb_sb, in_=temb_d[:])
        qb = loadb(qb_d, "qb")
        kb = loadb(kb_d, "kb")
        tb = loadb(tb_d, "tb")
        kwT = load3(kwT_d, N_KT, C, "kwT", F32R)
        qwT = load3(qwT_d, N_KT, C, "qwT", F32R)
        vwT = load3(vwT_d, N_KT, CV, "vwT", F32R)

        vb_bc = const.tile([128, CV], F32, tag="vb_bc")
        vb_ap = vb_d[:]
        nc.gpsimd.dma_start(
            out=vb_bc,
            in_=bass.AP(tensor=vb_ap.tensor, offset=vb_ap.offset,
                        ap=[[0, 128], [1, CV]]),
        )

        # x loaded in 512-col chunks so compute follows the DMA stream;
        # chunks 0..3 double as the query half and the residual.
        xk_ch = [[big.tile([128, 512], F32R, tag=f"xk{kt}_{ch}", name=f"xk_{kt}_{ch}")
                  for ch in range(N_CH)] for kt in range(N_KT)]
        dma_engines = [nc.sync]
        for ch in range(N_CH):
            for kt in range(N_KT):
                eng = dma_engines[(ch * N_KT + kt) % len(dma_engines)]
                r0 = ch * C + kt * 128
                eng.dma_start(out=xk_ch[kt][ch], in_=xk_d[r0:r0 + 128, :])

        pwT = load3(pwT_d, N_KT, C, "pwT", BF16)
        pb = loadb(pb_d, "pb")
        ident = const.tile([128, 128], BF16, tag="ident")
        nc.gpsimd.dma_start(out=ident, in_=id_d[:])

        # per-chunk K / V~T / Q tiles for fine-grained dependencies
        k_ch = [big.tile([128, N_KT, 512], BF16, tag=f"k{ch}", name=f"k_{ch}")
                for ch in range(N_CH)]
        vT_ch = [big.tile([128, 4, CV], BF16, tag=f"vT{ch}", name=f"vT_{ch}")
                 for ch in range(N_CH)]
        q_ch = [big.tile([128, N_KT, 512], BF16, tag=f"q{ib}", name=f"q_{ib}")
                for ib in range(N_IB)]
        qbias = const.tile([128, N_KT], F32, tag="qbias")
        kbias = const.tile([128, N_KT], F32, tag="kbias")

        with tc.tile_pool(name="aps", bufs=2, space="PSUM") as aps, \
             tc.tile_pool(name="hps", bufs=4, space="PSUM") as hps, \
             tc.tile_pool(name="ppool", bufs=8) as ppool, \
             tc.tile_pool(name="htpool", bufs=6) as htpool, \
             tc.tile_pool(name="hhpool", bufs=4) as hhpool, \
             tc.tile_pool(name="opool", bufs=4) as opool, \
             tc.tile_pool(name="rpool", bufs=8) as rpool:

            hh_ps_of = {}

            def emit_attn_jt(ib, jt):
                """One j-tile of attention for query block ib."""
                ch, jj = divmod(jt, 4)
                att = aps.tile([128, 512], F32, tag="w", name=f"att{ib}_{jt}")
                for kt in range(N_KT):
                    nc.tensor.matmul(
                        att,
                        lhsT=k_ch[ch][:, kt, jj * 128:(jj + 1) * 128],
                        rhs=q_ch[ib][:, kt, :],
                        start=(kt == 0),
                        stop=(kt == N_KT - 1),
                    )
                pt = ppool.tile([128, 512], BF16, tag="pT", name=f"pt{ib}_{jt}")
                nc.scalar.activation(out=pt, in_=att, func=AF.Exp)
                for isl in range(4):
                    nc.tensor.matmul(
                        hh_ps_of[ib][isl],
                        lhsT=pt[:, isl * 128:(isl + 1) * 128],
                        rhs=vT_ch[ch][:, jj, :],
                        start=(jt == 0),
                        stop=(jt == N_JT - 1),
                    )

            def emit_tail(ib, wk):
                """Normalize + transpose to channel-major for block ib."""
                hh_half = [hhpool.tile([128, 512], BF16, tag=f"hhsb{ct}",
                                       name=f"hh_half{ib}_{ct}")
                           for ct in range(N_KT)]
                for isl in range(4):
                    rc = rpool.tile([128, 1], F32, tag="rc", name=f"rc{ib}_{isl}")
                    nc.vector.reciprocal(rc, hh_ps_of[ib][isl][:, C:C + 1])
                    ht = htpool.tile([128, C], BF16, tag="ht", name=f"ht{ib}_{isl}")
                    nc.scalar.activation(out=ht, in_=hh_ps_of[ib][isl][:, 0:C],
                                         func=AF.Copy, scale=rc)
                    for ct in range(N_KT):
                        trp = wk.tile([128, 128], BF16, tag="wk",
                                      name=f"trp{ib}_{isl}_{ct}")
                        nc.tensor.transpose(
                            trp, ht[:, ct * 128:(ct + 1) * 128], ident)
                        nc.vector.tensor_copy(
                            hh_half[ct][:, isl * 128:(isl + 1) * 128], trp)
                return hh_half

            def emit_pconv(ib, hh_half, wk):
                """p-conv + bias + residual + store for query block ib."""
                i0 = ib * 512
                for ot in range(N_KT):
                    po = wk.tile([128, 512], F32, tag="wk", name=f"po{ib}_{ot}")
                    for ct in range(N_KT):
                        nc.tensor.matmul(
                            po,
                            lhsT=pwT[:, ct, ot * 128:(ot + 1) * 128],
                            rhs=hh_half[ct],
                            start=(ct == 0),
                            stop=(ct == N_KT - 1),
                        )
                    ob = opool.tile([128, 512], F32, tag="ob", name=f"ob{ib}_{ot}")
                    nc.vector.tensor_scalar_add(ob, po, pb[:, ot:ot + 1])
                    nc.vector.tensor_add(ob, ob, xk_ch[ot][ib].bitcast(F32))
                    nc.sync.dma_start(
                        out=out_d[ot * 128:(ot + 1) * 128, i0:i0 + 512], in_=ob
                    )

            # ---- conv phase, interleaved with query block 0's attention ----
            with tc.tile_pool(name="cps", bufs=2, space="PSUM") as cps, \
                 tc.tile_pool(name="tv", bufs=2) as tvp:
                # temb projection: t = t_w @ temb_b + t_b (per-partition)
                for mt in range(N_KT):
                    pt = cps.tile([128, 1], F32, tag="cv", name=f"tps{mt}")
                    for kt in range(TEMB // 128):
                        nc.tensor.matmul(
                            pt,
                            lhsT=twT[:, kt, mt * 128:(mt + 1) * 128],
                            rhs=temb_sb[:, kt:kt + 1],
                            start=(kt == 0),
                            stop=(kt == TEMB // 128 - 1),
                        )
                    tvec = tvp.tile([128, 1], F32, tag="tvec")
                    nc.vector.tensor_scalar_add(tvec, pt, tb[:, mt:mt + 1])
                    # qbias = (q_b + t) * scale  (q_w pre-scaled on host)
                    nc.vector.tensor_scalar(
                        qbias[:, mt:mt + 1], tvec, qb[:, mt:mt + 1], SCALE,
                        op0=mybir.AluOpType.add, op1=mybir.AluOpType.mult,
                    )
                    nc.vector.tensor_scalar_add(
                        kbias[:, mt:mt + 1], tvec, kb[:, mt:mt + 1])

                hh_ps_of[0] = [hps.tile([128, CV], F32, tag="hh",
                                        name=f"hh_ps0_{isl}") for isl in range(4)]
                for ch in range(N_CH):
                    for mt in range(N_KT):
                        ps = cps.tile([128, 512], F32, tag="cv", name=f"psk{mt}_{ch}")
                        for kt in range(N_KT):
                            nc.tensor.matmul(
                                ps,
                                lhsT=kwT[:, kt, mt * 128:(mt + 1) * 128],
                                rhs=xk_ch[kt][ch],
                                start=(kt == 0),
                                stop=(kt == N_KT - 1),
                            )
                        nc.vector.tensor_scalar_add(
                            k_ch[ch][:, mt, :], ps, kbias[:, mt:mt + 1])
                    if ch < N_IB:
                        for mt in range(N_KT):
                            ps = cps.tile([128, 512], F32, tag="cv",
                                          name=f"psq{mt}_{ch}")
                            for kt in range(N_KT):
                                nc.tensor.matmul(
                                    ps,
                                    lhsT=qwT[:, kt, mt * 128:(mt + 1) * 128],
                                    rhs=xk_ch[kt][ch],
                                    start=(kt == 0),
                                    stop=(kt == N_KT - 1),
                                )
                            nc.vector.tensor_scalar_add(
                                q_ch[ch][:, mt, :], ps, qbias[:, mt:mt + 1])
                    for jj in range(4):
                        jt = ch * 4 + jj
                        ps = cps.tile([128, CV], F32, tag="cv", name=f"psv{jt}")
                        for kt in range(N_KT):
                            nc.tensor.matmul(
                                ps,
                                lhsT=xk_ch[kt][ch][:, jj * 128:(jj + 1) * 128],
                                rhs=vwT[:, kt, :],
                                start=(kt == 0),
                                stop=(kt == N_KT - 1),
                            )
                        nc.vector.tensor_add(vT_ch[ch][:, jj, :], ps, vb_bc)
                    # trail one chunk behind the convs with block 0's attention
                    if ch >= 1:
                        for jt in range((ch - 1) * 4, ch * 4):
                            emit_attn_jt(0, jt)
                for jt in range((N_CH - 1) * 4, N_JT):
                    emit_attn_jt(0, jt)

            # ---- remaining query blocks ----
            with tc.tile_pool(name="wk", bufs=2, space="PSUM") as wk:
                pending = (0, emit_tail(0, wk))
                for ib in range(1, N_IB):
                    hh_ps_of[ib] = [hps.tile([128, CV], F32, tag="hh",
                                             name=f"hh_ps{ib}_{isl}")
                                    for isl in range(4)]
                    for jt in range(N_JT):
                        emit_attn_jt(ib, jt)
                        if pending is not None and jt == 4:
                            emit_pconv(pending[0], pending[1], wk)
                            pending = None
                    pending = (ib, emit_tail(ib, wk))
                emit_pconv(pending[0], pending[1], wk)

    nc.compile()
    return nc


_NC_CACHE = None


def _get_nc():
    global _NC_CACHE
    if _NC_CACHE is None:
        _NC_CACHE = build_nc()
    return _NC_CACHE


def make_in_maps(x, temb, q_w, q_b, k_w, k_b, v_w, v_b, p_w, p_b, t_w, t_b):
    xf = np.asarray(x, np.float32).reshape(B, C, HW)
    temb = np.asarray(temb, np.float32)
    bf16 = ml_dtypes.bfloat16
    vwT = np.concatenate(
        [np.asarray(v_w, np.float32).T, np.zeros((C, 2), np.float32)], axis=1)
    vb = np.concatenate(
        [np.asarray(v_b, np.float32).reshape(1, C),
         np.array([[1.0, 0.0]], np.float32)], axis=1)
    common = {
        "qwT": np.ascontiguousarray(np.asarray(q_w, np.float32).T * SCALE),
        "kwT": np.ascontiguousarray(np.asarray(k_w, np.float32).T),
        "vwT": np.ascontiguousarray(vwT),
        "pwT": np.ascontiguousarray(np.asarray(p_w, np.float32).T).astype(bf16),
        "twT": np.ascontiguousarray(np.asarray(t_w, np.float32).T).astype(bf16),
        "qb": np.asarray(q_b, np.float32).reshape(C, 1).copy(),
        "kb": np.asarray(k_b, np.float32).reshape(C, 1).copy(),
        "vb": vb,
        "pb": np.asarray(p_b, np.float32).reshape(C, 1).copy(),
        "tb": np.asarray(t_b, np.float32).reshape(C, 1).copy(),
        "ident": np.eye(128, dtype=bf16),
    }
    in_maps = []
    for core in range(N_CORES):
        b, h = divmod(core, 2)
        m = dict(common)
        # rotate so this core's query half occupies columns 0..NQ-1;
        # key order is irrelevant (softmax sums over keys).
        xr = xf[b] if h == 0 else np.concatenate(
            [xf[b][:, NQ:], xf[b][:, :NQ]], axis=1)
        # chunk-contiguous: [N_CH, C, 512] flattened, so each 256KB chunk
        # is one linear DRAM span (fast DMA descriptors)
        m["xk"] = np.ascontiguousarray(
            xr.reshape(C, N_CH, 512).transpose(1, 0, 2)).reshape(N_CH * C, 512)
        m["tembc"] = np.ascontiguousarray(
            temb[b].reshape(TEMB // 128, 128).T).astype(bf16)
        in_maps.append(m)
    return in_maps


def run(in_maps, trace=False):
    nc = _get_nc()
    return run_bass_kernel_spmd(nc, in_maps, core_ids=list(range(N_CORES)),
                                trace=trace)


def kernel(**inputs):
    in_maps = make_in_maps(**inputs)
    res = run(in_maps)
    out = np.empty((B, C, HW), np.float32)
    for core in range(N_CORES):
        b, h = divmod(core, 2)
        out[b, :, h * NQ:(h + 1) * NQ] = res.results[core]["out"]
    return out.reshape(B, C, H, W)



# revision 2
# speedup vs baseline: 1.2641x; 1.2641x over previous
"""AttentionBlock kernel for Trainium2, 8 NeuronCores.

Reference computation (B=4, C=256, H=W=64, TEMB=1024):
    t  = temb @ t_w.T + t_b                       # [B, C]
    q  = q_w @ x + (q_b + t)   (1x1 conv)         # [B, C, HW]
    k  = k_w @ x + (k_b + t)
    v  = v_w @ x + v_b
    att = softmax(q.T k / sqrt(C), axis=j)        # [B, HW, HW]
    hh  = att @ v.T                               # [B, C, HW]
    out = x + p_w @ hh + p_b

Sharding: data-parallel over (batch, query-half): core = b*2 + h.
Each core receives x[b] with its OWN query half rotated to the front
(keys may be processed in any order -- softmax is a sum over keys), so
one [C, HW] tensor serves the K/V convs, the Q conv (chunks 0-3) and
the residual.  The flash-attention-style kernel runs entirely in SBUF:
the 67M-entry attention matrix is never materialized to HBM.

Layout choices (evidence-driven from NTFF traces):
  - k bias (k_b + t) is dropped entirely: a per-channel shift of k adds
    a per-QUERY constant to every score row, which softmax normalizes
    away exactly (the ones-column denominator uses the same shifted
    exps).  The 1/sqrt(C) scale rides the exp instruction (exp(s*in)),
    so q/k stay at their natural ~N(0, 0.1) scale for fp8.
  - q, k, v and the exp'd attention weights are stored as fp8e4; both
    attention matmuls run in MatmulPerfMode.DoubleRow (two 128-row
    k-tiles contracted per instruction at 2 rows/cycle), quartering
    matmul-1 and halving matmul-2 PE time vs the f32r/bf16 baseline.
  - matmul 2: hhT[i, c] = sum_j p[j,i] vT[j,c] with fp8 p-slices as
    weights and fp8 vT pairs as the moving operand.  vT carries a ones
    column, so hhT[:, 256] = softmax denominator lands per-partition
    for free; normalization is a per-partition scale on the DVE.
  - ACT does nothing but exp (its throughput floor, free-dim elements
    at ~0.83ns each, is the kernel's critical path); the normalization
    scale-copies run on the DVE, and the final bias+residual is one
    fused scalar_tensor_tensor.
  - hhT -> channel-major hh via 8 bf16 PE transposes per query block
    (DMA transposes serialized ~1.2us each and stalled the PE).
  - block N's p-conv is deferred into block N+1's matmul stream, and
    query block 0's whole attention loop is interleaved with the convs
    chunk-by-chunk, so the PE chases the x DMA stream and the HAM clock
    gate never re-throttles mid-kernel.
"""

import numpy as np
import ml_dtypes
from contextlib import ExitStack

import concourse.bass as bass
import concourse.mybir as mybir
import concourse.tile as tile
from concourse import bacc
from concourse.bass_utils import run_bass_kernel_spmd

F32 = mybir.dt.float32
F32R = mybir.dt.float32r
BF16 = mybir.dt.bfloat16
FP8 = mybir.dt.float8e4
AF = mybir.ActivationFunctionType
DR = mybir.MatmulPerfMode.DoubleRow

B, C, H, W, TEMB = 4, 256, 64, 64, 1024
HW = H * W              # 4096
NQ = HW // 2            # 2048 query pixels per core
N_CORES = 8
SCALE = float(C) ** -0.5

N_CH = HW // 512        # 8 x-chunks of 512 columns
N_JT = HW // 128        # 32 key tiles of 128
N_PAIR = N_JT // 2      # 16 key-tile pairs (DoubleRow contracts 2 at once)
N_IB = NQ // 512        # 4 query blocks of 512
N_KT = C // 128         # 2 channel tiles
CV = C + 2              # vT width: 256 channels + ones col + pad


def build_nc():
    nc = bacc.Bacc("TRN2", target_bir_lowering=False, debug=False)

    # ---- DRAM I/O (per-core) ----
    xk_d = nc.dram_tensor("xk", [N_CH * C, 512], F32R, kind="ExternalInput")
    temb_d = nc.dram_tensor("tembc", [128, TEMB // 128], BF16, kind="ExternalInput")
    qwT_d = nc.dram_tensor("qwT", [C, C], F32R, kind="ExternalInput")
    kwT_d = nc.dram_tensor("kwT", [C, C], F32R, kind="ExternalInput")
    vwT_d = nc.dram_tensor("vwT", [C, CV], F32R, kind="ExternalInput")
    pwT_d = nc.dram_tensor("pwT", [C, C], BF16, kind="ExternalInput")
    twT_d = nc.dram_tensor("twT", [TEMB, C], BF16, kind="ExternalInput")
    qb_d = nc.dram_tensor("qb", [C, 1], F32, kind="ExternalInput")
    vb_d = nc.dram_tensor("vb", [1, CV], F32, kind="ExternalInput")
    pb_d = nc.dram_tensor("pb", [C, 1], F32, kind="ExternalInput")
    tb_d = nc.dram_tensor("tb", [C, 1], F32, kind="ExternalInput")
    id_d = nc.dram_tensor("ident", [128, 128], BF16, kind="ExternalInput")
    out_d = nc.dram_tensor("out", [C, NQ], F32, kind="ExternalOutput")

    with tile.TileContext(nc) as tc, ExitStack() as ctx:
        const = ctx.enter_context(tc.tile_pool(name="const", bufs=1))
        big = ctx.enter_context(tc.tile_pool(name="big", bufs=1))

        # ---- loads, in rough order of first use on the PE ----
        def load3(dram, kt_n, width, name, dt):
            t = const.tile([128, kt_n, width], dt, tag=name)
            nc.sync.dma_start(out=t, in_=dram[:].rearrange("(a p) o -> p a o", p=128))
            return t

        def loadb(dram, name):
            t = const.tile([128, N_KT], F32, tag=name)
            nc.sync.dma_start(
                out=t, in_=dram[:].rearrange("(a p) one -> p (a one)", p=128)
            )
            return t

        twT = load3(twT_d, TEMB // 128, C, "twT", BF16)
        temb_sb = const.tile([128, TEMB // 128], BF16, tag="temb")
        nc.sync.dma_start(out=temb_sb, in_=temb_d[:])
        qb = loadb(qb_d, "qb")
        tb = loadb(tb_d, "tb")
        kwT = load3(kwT_d, N_KT, C, "kwT", F32R)
        qwT = load3(qwT_d, N_KT, C, "qwT", F32R)
        vwT = load3(vwT_d, N_KT, CV, "vwT", F32R)

        vb_bc = const.tile([128, CV], F32, tag="vb_bc")
        vb_ap = vb_d[:]
        nc.gpsimd.dma_start(
            out=vb_bc,
            in_=bass.AP(tensor=vb_ap.tensor, offset=vb_ap.offset,
                        ap=[[0, 128], [1, CV]]),
        )

        # x loaded in 512-col chunks so compute follows the DMA stream;
        # chunks 0..3 double as the query half and the residual.
        xk_ch = [[big.tile([128, 512], F32R, tag=f"xk{kt}_{ch}", name=f"xk_{kt}_{ch}")
                  for ch in range(N_CH)] for kt in range(N_KT)]
        dma_engines = [nc.sync]
        for ch in range(N_CH):
            for kt in range(N_KT):
                eng = dma_engines[(ch * N_KT + kt) % len(dma_engines)]
                r0 = ch * C + kt * 128
                eng.dma_start(out=xk_ch[kt][ch], in_=xk_d[r0:r0 + 128, :])

        pwT = load3(pwT_d, N_KT, C, "pwT", BF16)
        pb = loadb(pb_d, "pb")
        ident = const.tile([128, 128], BF16, tag="ident")
        nc.gpsimd.dma_start(out=ident, in_=id_d[:])

        # per-chunk K / V~T / Q tiles (fp8) for fine-grained dependencies
        k_ch = [big.tile([128, N_KT, 512], FP8, tag=f"k{ch}", name=f"k_{ch}")
                for ch in range(N_CH)]
        vT_ch = [big.tile([128, 4, CV], FP8, tag=f"vT{ch}", name=f"vT_{ch}")
                 for ch in range(N_CH)]
        q_ch = [big.tile([128, N_KT, 512], FP8, tag=f"q{ib}", name=f"q_{ib}")
                for ib in range(N_IB)]
        qbias = const.tile([128, N_KT], F32, tag="qbias")

        with tc.tile_pool(name="aps", bufs=2, space="PSUM") as aps, \
             tc.tile_pool(name="hps", bufs=4, space="PSUM") as hps, \
             tc.tile_pool(name="ppool", bufs=4) as ppool, \
             tc.tile_pool(name="htpool", bufs=6) as htpool, \
             tc.tile_pool(name="hhpool", bufs=4) as hhpool, \
             tc.tile_pool(name="opool", bufs=4) as opool, \
             tc.tile_pool(name="rpool", bufs=8) as rpool:

            hh_ps_of = {}

            def emit_attn_pair(ib, pair):
                """One key-tile PAIR of attention for query block ib."""
                ch, pp = divmod(pair, 2)
                pt = ppool.tile([128, 2, 512], FP8, tag="pT",
                                name=f"pt{ib}_{pair}")
                for h in range(2):
                    jj = pp * 2 + h
                    att = aps.tile([128, 512], F32, tag="w",
                                   name=f"att{ib}_{pair}_{h}")
                    nc.tensor.matmul(
                        att,
                        lhsT=k_ch[ch][:, :, jj * 128:(jj + 1) * 128],
                        rhs=q_ch[ib][:, :, :],
                        start=True, stop=True, perf_mode=DR,
                    )
                    nc.scalar.activation(out=pt[:, h, :], in_=att,
                                         func=AF.Exp, scale=SCALE)
                for isl in range(4):
                    nc.tensor.matmul(
                        hh_ps_of[ib][isl],
                        lhsT=pt[:, :, isl * 128:(isl + 1) * 128],
                        rhs=vT_ch[ch][:, pp * 2:pp * 2 + 2, :],
                        start=(pair == 0),
                        stop=(pair == N_PAIR - 1),
                        perf_mode=DR,
                    )

            def emit_tail(ib, wk):
                """Normalize + transpose to channel-major for block ib."""
                hh_half = [hhpool.tile([128, 512], BF16, tag=f"hhsb{ct}",
                                       name=f"hh_half{ib}_{ct}")
                           for ct in range(N_KT)]
                for isl in range(4):
                    rc = rpool.tile([128, 1], F32, tag="rc", name=f"rc{ib}_{isl}")
                    nc.vector.reciprocal(rc, hh_ps_of[ib][isl][:, C:C + 1])
                    ht = htpool.tile([128, C], BF16, tag="ht", name=f"ht{ib}_{isl}")
                    nc.vector.tensor_scalar_mul(ht, hh_ps_of[ib][isl][:, 0:C], rc)
                    for ct in range(N_KT):
                        trp = wk.tile([128, 128], BF16, tag="wk",
                                      name=f"trp{ib}_{isl}_{ct}")
                        nc.tensor.transpose(
                            trp, ht[:, ct * 128:(ct + 1) * 128], ident)
                        nc.vector.tensor_copy(
                            hh_half[ct][:, isl * 128:(isl + 1) * 128], trp)
                return hh_half

            def emit_pconv(ib, hh_half, wk):
                """p-conv + bias + residual + store for query block ib."""
                i0 = ib * 512
                for ot in range(N_KT):
                    po = wk.tile([128, 512], F32, tag="wk", name=f"po{ib}_{ot}")
                    for ct in range(N_KT):
                        nc.tensor.matmul(
                            po,
                            lhsT=pwT[:, ct, ot * 128:(ot + 1) * 128],
                            rhs=hh_half[ct],
                            start=(ct == 0),
                            stop=(ct == N_KT - 1),
                        )
                    ob = opool.tile([128, 512], F32, tag="ob", name=f"ob{ib}_{ot}")
                    nc.vector.scalar_tensor_tensor(
                        ob, in0=po, scalar=pb[:, ot:ot + 1],
                        in1=xk_ch[ot][ib].bitcast(F32),
                        op0=mybir.AluOpType.add, op1=mybir.AluOpType.add,
                    )
                    nc.sync.dma_start(
                        out=out_d[ot * 128:(ot + 1) * 128, i0:i0 + 512], in_=ob
                    )

            # ---- conv phase, interleaved with query block 0's attention ----
            with tc.tile_pool(name="cps", bufs=2, space="PSUM") as cps, \
                 tc.tile_pool(name="tv", bufs=2) as tvp:
                # temb projection: qbias = (t_w @ temb_b + t_b) + q_b
                for mt in range(N_KT):
                    pt_ps = cps.tile([128, 1], F32, tag="cv", name=f"tps{mt}")
                    for kt in range(TEMB // 128):
                        nc.tensor.matmul(
                            pt_ps,
                            lhsT=twT[:, kt, mt * 128:(mt + 1) * 128],
                            rhs=temb_sb[:, kt:kt + 1],
                            start=(kt == 0),
                            stop=(kt == TEMB // 128 - 1),
                        )
                    nc.vector.tensor_scalar(
                        qbias[:, mt:mt + 1], pt_ps, tb[:, mt:mt + 1],
                        qb[:, mt:mt + 1],
                        op0=mybir.AluOpType.add, op1=mybir.AluOpType.add,
                    )

                hh_ps_of[0] = [hps.tile([128, CV], F32, tag="hh",
                                        name=f"hh_ps0_{isl}") for isl in range(4)]
                for ch in range(N_CH):
                    for mt in range(N_KT):
                        ps = cps.tile([128, 512], F32, tag="cv", name=f"psk{mt}_{ch}")
                        for kt in range(N_KT):
                            nc.tensor.matmul(
                                ps,
                                lhsT=kwT[:, kt, mt * 128:(mt + 1) * 128],
                                rhs=xk_ch[kt][ch],
                                start=(kt == 0),
                                stop=(kt == N_KT - 1),
                            )
                        nc.vector.tensor_copy(k_ch[ch][:, mt, :], ps)
                    if ch < N_IB:
                        for mt in range(N_KT):
                            ps = cps.tile([128, 512], F32, tag="cv",
                                          name=f"psq{mt}_{ch}")
                            for kt in range(N_KT):
                                nc.tensor.matmul(
                                    ps,
                                    lhsT=qwT[:, kt, mt * 128:(mt + 1) * 128],
                                    rhs=xk_ch[kt][ch],
                                    start=(kt == 0),
                                    stop=(kt == N_KT - 1),
                                )
                            nc.vector.tensor_scalar_add(
                                q_ch[ch][:, mt, :], ps, qbias[:, mt:mt + 1])
                    for jj in range(4):
                        jt = ch * 4 + jj
                        ps = cps.tile([128, CV], F32, tag="cv", name=f"psv{jt}")
                        for kt in range(N_KT):
                            nc.tensor.matmul(
                                ps,
                                lhsT=xk_ch[kt][ch][:, jj * 128:(jj + 1) * 128],
                                rhs=vwT[:, kt, :],
                                start=(kt == 0),
                                stop=(kt == N_KT - 1),
                            )
                        nc.vector.tensor_add(vT_ch[ch][:, jj, :], ps, vb_bc)
                    # trail one chunk behind the convs with block 0's attention
                    if ch >= 1:
                        for pair in range(2 * (ch - 1), 2 * ch):
                            emit_attn_pair(0, pair)
                for pair in range(2 * (N_CH - 1), N_PAIR):
                    emit_attn_pair(0, pair)

            # ---- remaining query blocks ----
            with tc.tile_pool(name="wk", bufs=2, space="PSUM") as wk:
                pending = (0, emit_tail(0, wk))
                for ib in range(1, N_IB):
                    hh_ps_of[ib] = [hps.tile([128, CV], F32, tag="hh",
                                             name=f"hh_ps{ib}_{isl}")
                                    for isl in range(4)]
                    for pair in range(N_PAIR):
                        emit_attn_pair(ib, pair)
                        if pending is not None and pair == 2:
                            emit_pconv(pending[0], pending[1], wk)
                            pending = None
                    pending = (ib, emit_tail(ib, wk))
                emit_pconv(pending[0], pending[1], wk)

    nc.compile()
    return nc


_NC_CACHE = None


def _get_nc():
    global _NC_CACHE
    if _NC_CACHE is None:
        _NC_CACHE = build_nc()
    return _NC_CACHE


def make_in_maps(x, temb, q_w, q_b, k_w, k_b, v_w, v_b, p_w, p_b, t_w, t_b):
    xf = np.asarray(x, np.float32).reshape(B, C, HW)
    temb = np.asarray(temb, np.float32)
    bf16 = ml_dtypes.bfloat16
    vwT = np.concatenate(
        [np.asarray(v_w, np.float32).T, np.zeros((C, 2), np.float32)], axis=1)
    vb = np.concatenate(
        [np.asarray(v_b, np.float32).reshape(1, C),
         np.array([[1.0, 0.0]], np.float32)], axis=1)
    common = {
        "qwT": np.ascontiguousarray(np.asarray(q_w, np.float32).T),
        "kwT": np.ascontiguousarray(np.asarray(k_w, np.float32).T),
        "vwT": np.ascontiguousarray(vwT),
        "pwT": np.ascontiguousarray(np.asarray(p_w, np.float32).T).astype(bf16),
        "twT": np.ascontiguousarray(np.asarray(t_w, np.float32).T).astype(bf16),
        "qb": np.asarray(q_b, np.float32).reshape(C, 1).copy(),
        "vb": vb,
        "pb": np.asarray(p_b, np.float32).reshape(C, 1).copy(),
        "tb": np.asarray(t_b, np.float32).reshape(C, 1).copy(),
        "ident": np.eye(128, dtype=bf16),
    }
    in_maps = []
    for core in range(N_CORES):
        b, h = divmod(core, 2)
        m = dict(common)
        # rotate so this core's query half occupies columns 0..NQ-1;
        # key order is irrelevant (softmax sums over keys).
        xr = xf[b] if h == 0 else np.concatenate(
            [xf[b][:, NQ:], xf[b][:, :NQ]], axis=1)
        # chunk-contiguous: [N_CH, C, 512] flattened, so each 256KB chunk
        # is one linear DRAM span (fast DMA descriptors)
        m["xk"] = np.ascontiguousarray(
            xr.reshape(C, N_CH, 512).transpose(1, 0, 2)).reshape(N_CH * C, 512)
        m["tembc"] = np.ascontiguousarray(
            temb[b].reshape(TEMB // 128, 128).T).astype(bf16)
        in_maps.append(m)
    return in_maps


def run(in_maps, trace=False):
    nc = _get_nc()
    return run_bass_kernel_spmd(nc, in_maps, core_ids=list(range(N_CORES)),
                                trace=trace)


def kernel(**inputs):
    in_maps = make_in_maps(**inputs)
    res = run(in_maps)
    out = np.empty((B, C, HW), np.float32)
    for core in range(N_CORES):
        b, h = divmod(core, 2)
        out[b, :, h * NQ:(h + 1) * NQ] = res.results[core]["out"]
    return out.reshape(B, C, H, W)


# revision 4
# speedup vs baseline: 1.3170x; 1.0418x over previous
"""AttentionBlock kernel for Trainium2, 8 NeuronCores.

Reference computation (B=4, C=256, H=W=64, TEMB=1024):
    t  = temb @ t_w.T + t_b                       # [B, C]
    q  = q_w @ x + (q_b + t)   (1x1 conv)         # [B, C, HW]
    k  = k_w @ x + (k_b + t)
    v  = v_w @ x + v_b
    att = softmax(q.T k / sqrt(C), axis=j)        # [B, HW, HW]
    hh  = att @ v.T                               # [B, C, HW]
    out = x + p_w @ hh + p_b

Sharding: data-parallel over (batch, query-half): core = b*2 + h.
Each core receives x[b] with its OWN query half rotated to the front
(keys may be processed in any order -- softmax is a sum over keys).
The flash-attention-style kernel runs entirely in SBUF: the 67M-entry
attention matrix is never materialized to HBM.

Layout choices (evidence-driven from NTFF traces):
  - k bias (k_b + t) is dropped entirely: a per-channel shift of k adds
    a per-QUERY constant to every score row, which softmax normalizes
    away exactly (the ones-column denominator uses the same shifted
    exps).
  - everything up to the attention output runs in fp8e4 with
    MatmulPerfMode.DoubleRow (contract two 128-row k-tiles per
    instruction at 2 rows/cycle).  Measured DR matmul cost is
    ~165ns fixed + 1 cycle per output row, so wide (512) moving
    operands amortize the fixed cost.
  - conv weights are pre-scaled by 64 on the host so their ~0.02-scale
    entries land in fp8e4's normal range; q/k/v then carry a 64x scale
    that cancels in softmax (the ones column is 64, so the denominator
    is 64*sum(p) and the normalization step yields natural-scale hh).
    The combined 1/(64*64*sqrt(C)) rides the exp instruction's input
    scale for free.  x ships as fp8 (1MB/core) for the convs plus an
    f32 residual half (2MB) pre-biased with p_b on the host.
  - matmul 2: hhT[i, c] = sum_j p[j,i] vT[j,c] with fp8 p-slices as
    weights and fp8 vT pairs as the moving operand; vT's 64s-column
    makes hhT[:, 256] the softmax denominator.
  - ACT does nothing but exp (its throughput floor, free-dim elements
    at ~0.83ns each, is the kernel's critical path); normalization
    scale-copies run on the DVE, k-tile PSUM->fp8 copies on the Pool
    engine, and the final residual add is one tensor_add.
  - hhT -> channel-major hh via 8 bf16 PE transposes per query block.
  - fp8 x and weights ride one DMA queue (sync) while the bf16
    temb/table constants ride the Pool queue, so the first conv starts
    ~1us in; block 0's attention interleaves with the convs
    chunk-by-chunk and block N's p-conv is deferred into block N+1's
    matmul stream so the PE never idles mid-kernel.
"""

import numpy as np
import ml_dtypes
from contextlib import ExitStack

import concourse.bass as bass
import concourse.mybir as mybir
import concourse.tile as tile
from concourse import bacc
from concourse.bass_utils import run_bass_kernel_spmd

F32 = mybir.dt.float32
BF16 = mybir.dt.bfloat16
FP8 = mybir.dt.float8e4
AF = mybir.ActivationFunctionType
DR = mybir.MatmulPerfMode.DoubleRow

B, C, H, W, TEMB = 4, 256, 64, 64, 1024
HW = H * W              # 4096
NQ = HW // 2            # 2048 query pixels per core
N_CORES = 8
WS = 64.0               # fp8 weight pre-scale
ESCALE = (float(C) ** -0.5) / (WS * WS)   # rides the exp instruction

N_CH = HW // 512        # 8 x-chunks of 512 columns
N_JT = HW // 128        # 32 key tiles of 128
N_PAIR = N_JT // 2      # 16 key-tile pairs (DoubleRow contracts 2 at once)
N_IB = NQ // 512        # 4 query blocks of 512
N_KT = C // 128         # 2 channel tiles
CV = C + 2              # vT width: 256 channels + 64s col + pad


def build_nc():
    nc = bacc.Bacc("TRN2", target_bir_lowering=False, debug=False)

    # ---- DRAM I/O (per-core) ----
    x8_d = nc.dram_tensor("x8", [N_CH * C, 512], FP8, kind="ExternalInput")
    xr_d = nc.dram_tensor("xr", [C, NQ], F32, kind="ExternalInput")
    temb_d = nc.dram_tensor("tembc", [128, TEMB // 128], BF16, kind="ExternalInput")
    qwT_d = nc.dram_tensor("qwT", [C, C], FP8, kind="ExternalInput")
    kwT_d = nc.dram_tensor("kwT", [C, C], FP8, kind="ExternalInput")
    vwT_d = nc.dram_tensor("vwT", [C, CV], FP8, kind="ExternalInput")
    pwT_d = nc.dram_tensor("pwT", [C, C], BF16, kind="ExternalInput")
    twT_d = nc.dram_tensor("twT", [TEMB, C], BF16, kind="ExternalInput")
    qb_d = nc.dram_tensor("qb", [C, 1], F32, kind="ExternalInput")
    vb_d = nc.dram_tensor("vb", [1, CV], F32, kind="ExternalInput")
    tb_d = nc.dram_tensor("tb", [C, 1], F32, kind="ExternalInput")
    id_d = nc.dram_tensor("ident", [128, 128], BF16, kind="ExternalInput")
    out_d = nc.dram_tensor("out", [C, NQ], F32, kind="ExternalOutput")

    with tile.TileContext(nc) as tc, ExitStack() as ctx:
        const = ctx.enter_context(tc.tile_pool(name="const", bufs=1))
        big = ctx.enter_context(tc.tile_pool(name="big", bufs=1))

        def load3(dram, kt_n, width, name, dt, eng):
            t = const.tile([128, kt_n, width], dt, tag=name)
            eng.dma_start(out=t, in_=dram[:].rearrange("(a p) o -> p a o", p=128))
            return t

        def loadb(dram, name):
            t = const.tile([128, N_KT], F32, tag=name)
            nc.gpsimd.dma_start(
                out=t, in_=dram[:].rearrange("(a p) one -> p (a one)", p=128)
            )
            return t

        # queue A (sync): the fp8 conv stream, then the f32 residual
        kwT = load3(kwT_d, N_KT, C, "kwT", FP8, nc.sync)
        qwT = load3(qwT_d, N_KT, C, "qwT", FP8, nc.sync)
        vwT = load3(vwT_d, N_KT, CV, "vwT", FP8, nc.sync)
        x8_ch = [big.tile([128, N_KT, 512], FP8, tag=f"x8_{ch}", name=f"x8_{ch}")
                 for ch in range(N_CH)]
        for ch in range(N_CH):
            nc.sync.dma_start(
                out=x8_ch[ch],
                in_=x8_d[ch * C:(ch + 1) * C, :].rearrange(
                    "(a p) o -> p a o", p=128),
            )
        xr_ch = [[big.tile([128, 512], F32, tag=f"xr{kt}_{ib}",
                           name=f"xr_{kt}_{ib}") for ib in range(N_IB)]
                 for kt in range(N_KT)]
        for ib in range(N_IB):
            for kt in range(N_KT):
                nc.sync.dma_start(
                    out=xr_ch[kt][ib],
                    in_=xr_d[kt * 128:(kt + 1) * 128, ib * 512:(ib + 1) * 512])

        # queue B (gpsimd): temb path + tail constants
        twT = load3(twT_d, TEMB // 128, C, "twT", BF16, nc.gpsimd)
        temb_sb = const.tile([128, TEMB // 128], BF16, tag="temb")
        nc.gpsimd.dma_start(out=temb_sb, in_=temb_d[:])
        qb = loadb(qb_d, "qb")
        tb = loadb(tb_d, "tb")
        vb_bc = const.tile([128, CV], F32, tag="vb_bc")
        vb_ap = vb_d[:]
        nc.gpsimd.dma_start(
            out=vb_bc,
            in_=bass.AP(tensor=vb_ap.tensor, offset=vb_ap.offset,
                        ap=[[0, 128], [1, CV]]),
        )
        pwT = load3(pwT_d, N_KT, C, "pwT", BF16, nc.gpsimd)
        ident = const.tile([128, 128], BF16, tag="ident")
        nc.gpsimd.dma_start(out=ident, in_=id_d[:])

        # per-chunk K / V~T / Q tiles (fp8) for fine-grained dependencies
        k_ch = [big.tile([128, N_KT, 512], FP8, tag=f"k{ch}", name=f"k_{ch}")
                for ch in range(N_CH)]
        vT_ch = [big.tile([128, 4, CV], FP8, tag=f"vT{ch}", name=f"vT_{ch}")
                 for ch in range(N_CH)]
        q_ch = [big.tile([128, N_KT, 512], FP8, tag=f"q{ib}", name=f"q_{ib}")
                for ib in range(N_IB)]
        qbias = const.tile([128, N_KT], F32, tag="qbias")

        with tc.tile_pool(name="aps", bufs=2, space="PSUM") as aps, \
             tc.tile_pool(name="hps", bufs=4, space="PSUM") as hps, \
             tc.tile_pool(name="ppool", bufs=4) as ppool, \
             tc.tile_pool(name="htpool", bufs=6) as htpool, \
             tc.tile_pool(name="hhpool", bufs=4) as hhpool, \
             tc.tile_pool(name="opool", bufs=4) as opool, \
             tc.tile_pool(name="rpool", bufs=8) as rpool:

            hh_ps_of = {}

            def emit_attn_pair(ib, pair):
                """One key-tile PAIR of attention for query block ib."""
                ch, pp = divmod(pair, 2)
                pt = ppool.tile([128, 2, 512], FP8, tag="pT",
                                name=f"pt{ib}_{pair}")
                for h in range(2):
                    jj = pp * 2 + h
                    att = aps.tile([128, 512], F32, tag="w",
                                   name=f"att{ib}_{pair}_{h}")
                    nc.tensor.matmul(
                        att,
                        lhsT=k_ch[ch][:, :, jj * 128:(jj + 1) * 128],
                        rhs=q_ch[ib][:, :, :],
                        start=True, stop=True, perf_mode=DR,
                    )
                    nc.scalar.activation(out=pt[:, h, :], in_=att,
                                         func=AF.Exp, scale=ESCALE)
                for isl in range(4):
                    nc.tensor.matmul(
                        hh_ps_of[ib][isl],
                        lhsT=pt[:, :, isl * 128:(isl + 1) * 128],
                        rhs=vT_ch[ch][:, pp * 2:pp * 2 + 2, :],
                        start=(pair == 0),
                        stop=(pair == N_PAIR - 1),
                        perf_mode=DR,
                    )

            def emit_tail(ib, wk):
                """Normalize + transpose to channel-major for block ib."""
                hh_half = [hhpool.tile([128, 512], BF16, tag=f"hhsb{ct}",
                                       name=f"hh_half{ib}_{ct}")
                           for ct in range(N_KT)]
                for isl in range(4):
                    rc = rpool.tile([128, 1], F32, tag="rc", name=f"rc{ib}_{isl}")
                    nc.vector.reciprocal(rc, hh_ps_of[ib][isl][:, C:C + 1])
                    ht = htpool.tile([128, C], BF16, tag="ht", name=f"ht{ib}_{isl}")
                    nc.vector.tensor_scalar_mul(ht, hh_ps_of[ib][isl][:, 0:C], rc)
                    for ct in range(N_KT):
                        trp = wk.tile([128, 128], BF16, tag="wk",
                                      name=f"trp{ib}_{isl}_{ct}")
                        nc.tensor.transpose(
                            trp, ht[:, ct * 128:(ct + 1) * 128], ident)
                        nc.vector.tensor_copy(
                            hh_half[ct][:, isl * 128:(isl + 1) * 128], trp)
                return hh_half

            def emit_pconv(ib, hh_half, wk):
                """p-conv + residual (pre-biased with p_b) + store."""
                i0 = ib * 512
                for ot in range(N_KT):
                    po = wk.tile([128, 512], F32, tag="wk", name=f"po{ib}_{ot}")
                    for ct in range(N_KT):
                        nc.tensor.matmul(
                            po,
                            lhsT=pwT[:, ct, ot * 128:(ot + 1) * 128],
                            rhs=hh_half[ct],
                            start=(ct == 0),
                            stop=(ct == N_KT - 1),
                        )
                    ob = opool.tile([128, 512], F32, tag="ob", name=f"ob{ib}_{ot}")
                    nc.vector.tensor_add(ob, po, xr_ch[ot][ib])
                    nc.sync.dma_start(
                        out=out_d[ot * 128:(ot + 1) * 128, i0:i0 + 512], in_=ob
                    )

            # ---- conv phase, interleaved with query block 0's attention ----
            with tc.tile_pool(name="cps", bufs=2, space="PSUM") as cps:
                # temb projection: qbias = 64*((t_w @ temb_b + t_b) + q_b)
                for mt in range(N_KT):
                    pt_ps = cps.tile([128, 1], F32, tag="cv", name=f"tps{mt}")
                    for kt in range(TEMB // 128):
                        nc.tensor.matmul(
                            pt_ps,
                            lhsT=twT[:, kt, mt * 128:(mt + 1) * 128],
                            rhs=temb_sb[:, kt:kt + 1],
                            start=(kt == 0),
                            stop=(kt == TEMB // 128 - 1),
                        )
                    nc.vector.tensor_scalar(
                        qbias[:, mt:mt + 1], pt_ps, tb[:, mt:mt + 1],
                        qb[:, mt:mt + 1],
                        op0=mybir.AluOpType.add, op1=mybir.AluOpType.add,
                    )

                hh_ps_of[0] = [hps.tile([128, CV], F32, tag="hh",
                                        name=f"hh_ps0_{isl}") for isl in range(4)]
                for ch in range(N_CH):
                    for mt in range(N_KT):
                        ps = cps.tile([128, 512], F32, tag="cv", name=f"psk{mt}_{ch}")
                        nc.tensor.matmul(
                            ps,
                            lhsT=kwT[:, :, mt * 128:(mt + 1) * 128],
                            rhs=x8_ch[ch][:, :, :],
                            start=True, stop=True, perf_mode=DR,
                        )
                        nc.vector.tensor_copy(k_ch[ch][:, mt, :], ps)
                    if ch < N_IB:
                        for mt in range(N_KT):
                            ps = cps.tile([128, 512], F32, tag="cv",
                                          name=f"psq{mt}_{ch}")
                            nc.tensor.matmul(
                                ps,
                                lhsT=qwT[:, :, mt * 128:(mt + 1) * 128],
                                rhs=x8_ch[ch][:, :, :],
                                start=True, stop=True, perf_mode=DR,
                            )
                            nc.vector.tensor_scalar_add(
                                q_ch[ch][:, mt, :], ps, qbias[:, mt:mt + 1])
                    for jj in range(4):
                        ps = cps.tile([128, CV], F32, tag="cv",
                                      name=f"psv{ch}_{jj}")
                        nc.tensor.matmul(
                            ps,
                            lhsT=x8_ch[ch][:, :, jj * 128:(jj + 1) * 128],
                            rhs=vwT[:, :, :],
                            start=True, stop=True, perf_mode=DR,
                        )
                        nc.vector.tensor_add(vT_ch[ch][:, jj, :], ps, vb_bc)
                    # trail one chunk behind the convs with block 0's attention
                    if ch >= 1:
                        for pair in range(2 * (ch - 1), 2 * ch):
                            emit_attn_pair(0, pair)
                for pair in range(2 * (N_CH - 1), N_PAIR):
                    emit_attn_pair(0, pair)

            # ---- remaining query blocks ----
            with tc.tile_pool(name="wk", bufs=2, space="PSUM") as wk:
                pending = (0, emit_tail(0, wk))
                for ib in range(1, N_IB):
                    hh_ps_of[ib] = [hps.tile([128, CV], F32, tag="hh",
                                             name=f"hh_ps{ib}_{isl}")
                                    for isl in range(4)]
                    for pair in range(N_PAIR):
                        emit_attn_pair(ib, pair)
                        if pending is not None and pair == 2:
                            emit_pconv(pending[0], pending[1], wk)
                            pending = None
                    pending = (ib, emit_tail(ib, wk))
                emit_pconv(pending[0], pending[1], wk)

    nc.compile()
    return nc


_NC_CACHE = None


def _get_nc():
    global _NC_CACHE
    if _NC_CACHE is None:
        _NC_CACHE = build_nc()
    return _NC_CACHE


def make_in_maps(x, temb, q_w, q_b, k_w, k_b, v_w, v_b, p_w, p_b, t_w, t_b):
    xf = np.asarray(x, np.float32).reshape(B, C, HW)
    temb = np.asarray(temb, np.float32)
    bf16 = ml_dtypes.bfloat16
    fp8 = ml_dtypes.float8_e4m3
    vwT = np.concatenate(
        [np.asarray(v_w, np.float32).T * WS, np.zeros((C, 2), np.float32)],
        axis=1)
    vb = np.concatenate(
        [np.asarray(v_b, np.float32).reshape(1, C) * WS,
         np.array([[WS, 0.0]], np.float32)], axis=1)
    common = {
        "qwT": np.ascontiguousarray(np.asarray(q_w, np.float32).T * WS).astype(fp8),
        "kwT": np.ascontiguousarray(np.asarray(k_w, np.float32).T * WS).astype(fp8),
        "vwT": np.ascontiguousarray(vwT).astype(fp8),
        "pwT": np.ascontiguousarray(np.asarray(p_w, np.float32).T).astype(bf16),
        "twT": np.ascontiguousarray(np.asarray(t_w, np.float32).T * WS).astype(bf16),
        "qb": np.asarray(q_b, np.float32).reshape(C, 1) * WS,
        "vb": vb,
        "tb": np.asarray(t_b, np.float32).reshape(C, 1) * WS,
        "ident": np.eye(128, dtype=bf16),
    }
    pbc = np.asarray(p_b, np.float32).reshape(C, 1)
    in_maps = []
    for core in range(N_CORES):
        b, h = divmod(core, 2)
        m = dict(common)
        # rotate so this core's query half occupies columns 0..NQ-1;
        # key order is irrelevant (softmax sums over keys).
        xr = xf[b] if h == 0 else np.concatenate(
            [xf[b][:, NQ:], xf[b][:, :NQ]], axis=1)
        # chunk-contiguous: [N_CH, C, 512] flattened, so each 128KB chunk
        # is one linear DRAM span (fast DMA descriptors)
        m["x8"] = np.ascontiguousarray(
            xr.reshape(C, N_CH, 512).transpose(1, 0, 2)).reshape(
                N_CH * C, 512).astype(fp8)
        m["xr"] = xr[:, :NQ] + pbc          # residual pre-biased with p_b
        m["tembc"] = np.ascontiguousarray(
            temb[b].reshape(TEMB // 128, 128).T).astype(bf16)
        in_maps.append(m)
    return in_maps


def run(in_maps, trace=False):
    nc = _get_nc()
    return run_bass_kernel_spmd(nc, in_maps, core_ids=list(range(N_CORES)),
                                trace=trace)


def kernel(**inputs):
    in_maps = make_in_maps(**inputs)
    res = run(in_maps)
    out = np.empty((B, C, HW), np.float32)
    for core in range(N_CORES):
        b, h = divmod(core, 2)
        out[b, :, h * NQ:(h + 1) * NQ] = res.results[core]["out"]
    return out.reshape(B, C, H, W)
